# revision 6
# baseline (speedup 1.0000x reference)
"""DepthCNN2d Trainium2 kernel.

Math (per batch b):
    depth = 2*(g_buffer[b,1] - 0.5)                       (H, W)
    dunf_k = 3x3 zero-padded shifts of depth              (9, L)
    e = exp(-(dunf - depth)^2); wf = 9*e / sum_k e        (9, L)
    filt[c*9+k, l] = x_k[c, l] * wf[k, l]                 (576, L)
    out[o, l] = sum_ck W2[o, ck] * filt[ck, l] + bias[o]  (128, L)

Device strategy (data-parallel over batch, one batch per NeuronCore):
  - wf computed compactly on 64 partitions (l-blocked layout), then
    rearranged to a (9, L) k-on-partition tile via 9 SBUF-SBUF DMAs.
  - wf broadcast across the 64 channel partitions with small selector
    matmuls on the PE (K=9) into PSUM, one per 128-row contraction chunk.
  - The 576-deep contraction is regrouped host-side into 5 chunks of
    (2 kernel positions x 64 channels); pairs are chosen with a constant
    +1 shift delta so a pre-shifted upper copy of the padded input lets
    a single 128-partition DVE multiply build each GEMM rhs tile.
  - 5 accumulating K=128 matmuls per 512-wide L-tile produce the output.
"""

import os

import numpy as np

import concourse.bacc as bacc
import concourse.bass as bass
import concourse.mybir as mybir
import concourse.tile as tile
from concourse.bass_utils import run_bass_kernel_spmd

N_CORES = 8
B, C, H, W = 8, 64, 128, 128
L = H * W            # 16384
COUT = 128
KH = KW = 3
PADW = W + 2         # 130
XPAD_F = 17024       # 130*130 padded out to 128*133 (slop for shifted reads)
NTILES = L // 512    # 32

# chunk pairing: (k0, k1) with shift delta +1 for the first three pairs,
# (2,5) handled by two half-ops, (8, zero) padded with a zero column.
PAIRS = [(0, 1), (3, 4), (6, 7), (2, 5), (8, None)]
# padded-buffer offset of kernel position k = (ky, kx):  ky*130 + kx
KOFF = [ky * PADW + kx for ky in range(3) for kx in range(3)]

GEMM_MODE = os.environ.get("DEPTHCNN_GEMM_MODE", "bf16")  # bf16 | fp32r | fp32


def _gemm_dt():
    if GEMM_MODE == "bf16":
        return mybir.dt.bfloat16
    if GEMM_MODE == "fp32r":
        return mybir.dt.float32r
    return mybir.dt.float32


def build_nc(reps=1):
    """Build the single-core SPMD program (same BIR on all 8 cores).

    reps > 1 wraps the whole computation in a hardware loop — used only
    for wall-clock timing (amortizes host/dispatch overhead).
    """
    gdt = _gemm_dt()
    f32 = mybir.dt.float32

    nc = bacc.Bacc("TRN2", target_bir_lowering=False, debug=False,
                   num_devices=N_CORES)

    x_dram = nc.dram_tensor("x", [C, H, W], f32, kind="ExternalInput")
    d_dram = nc.dram_tensor("d", [H, W], f32, kind="ExternalInput")
    wk_dram = nc.dram_tensor("wk", [5, 128, COUT], gdt, kind="ExternalInput")
    ssel_dram = nc.dram_tensor("ssel", [5, 9, 128], gdt, kind="ExternalInput")
    bias_dram = nc.dram_tensor("bias", [COUT], f32, kind="ExternalInput")
    out_dram = nc.dram_tensor("out", [COUT, L], f32, kind="ExternalOutput")

    with tile.TileContext(nc) as tc:
        with (
            tc.tile_pool(name="fixed", bufs=1) as fixed,
            tc.tile_pool(name="dram", bufs=1, space="DRAM") as dram,
            tc.tile_pool(name="pwf", bufs=5, space="PSUM") as pwf_pool,
            tc.tile_pool(name="pout", bufs=2, space="PSUM") as pout_pool,
            tc.tile_pool(name="filtp", bufs=6) as filt_pool,
            tc.tile_pool(name="osbp", bufs=3) as osb_pool,
        ):
          def body():
            # ---- static operands -------------------------------------
            wk_sb = fixed.tile([128, 5 * COUT], gdt)
            nc.sync.dma_start(
                wk_sb[:],
                bass.AP(wk_dram, 0, [[COUT, 128], [128 * COUT, 5], [1, COUT]]),
            )
            ssel_sb = fixed.tile([9, 5 * 128], gdt)
            nc.sync.dma_start(
                ssel_sb[:],
                bass.AP(ssel_dram, 0, [[128, 9], [9 * 128, 5], [1, 128]]),
            )
            bias_sb = fixed.tile([COUT, 1], f32)
            nc.sync.dma_start(bias_sb[:], bass.AP(bias_dram, 0, [[1, COUT], [1, 1]]))

            # ---- padded depth in DRAM --------------------------------
            dpad = dram.tile([XPAD_F], f32)
            zbuf = fixed.tile([128, 133], f32)
            nc.vector.memset(zbuf[:], 0.0)
            nc.sync.dma_start(dpad[:], zbuf[:])

            dsb = fixed.tile([H, W], f32)
            nc.sync.dma_start(dsb[:], d_dram.ap())
            dsc = fixed.tile([H, W], f32)
            # depth = 2*g - 1
            nc.scalar.activation(dsc[:], dsb[:], mybir.ActivationFunctionType.Copy,
                                 bias=-1.0, scale=2.0)
            dpad_int = dpad[PADW + 1:PADW + 1 + 128 * PADW].rearrange(
                "(a b) -> a b", b=PADW)[:, 0:W]
            nc.sync.dma_start(dpad_int, dsc[:])

            # ---- padded input in SBUF --------------------------------
            # partitions 0-63:  xpad[c]
            # partitions 64-127: xpad[c] shifted by +1 (value(64+c,a)=value(c,a+1))
            xpadA = fixed.tile([128, XPAD_F], f32)
            nc.vector.memset(xpadA[:], 0.0)
            xa_lo_int = xpadA[0:64, PADW + 1:PADW + 1 + 128 * PADW].rearrange(
                "p (a b) -> p a b", b=PADW)[:, :, 0:W]
            nc.sync.dma_start(xa_lo_int, x_dram.ap())
            xa_hi_dst = xpadA[64:128, PADW:PADW + 128 * PADW].rearrange(
                "p (a b) -> p a b", b=PADW)[:, :, 0:W]
            xa_lo_src = xpadA[0:64, PADW + 1:PADW + 1 + 128 * PADW].rearrange(
                "p (a b) -> p a b", b=PADW)[:, :, 0:W]
            nc.sync.dma_start(xa_hi_dst, xa_lo_src)

            # ---- bilateral weights ------------------------------------
            # l-blocked layout: partition p = l//256, free f = l%256
            # src pattern from dpad for kernel offset o: [[260,64],[130,2],[1,128]]
            def dpad_src(o):
                return bass.AP(dpad.tensor, dpad.offset + o,
                               [[2 * PADW, 64], [PADW, 2], [1, W]])

            dunf4 = fixed.tile([64, 9 * 256], f32)
            for k in range(9):
                dst = dunf4[:, k * 256:(k + 1) * 256].rearrange(
                    "p (a b) -> p a b", b=128)
                nc.sync.dma_start(dst, dpad_src(KOFF[k]))
            cent = fixed.tile([64, 256], f32)
            nc.sync.dma_start(
                cent[:].rearrange("p (a b) -> p a b", b=128), dpad_src(KOFF[4]))

            e4 = fixed.tile([64, 9 * 256], f32)
            for k in range(9):
                nc.vector.tensor_sub(e4[:, k * 256:(k + 1) * 256],
                                     dunf4[:, k * 256:(k + 1) * 256], cent[:])
            nc.scalar.square(e4[:], e4[:])
            nc.scalar.activation(e4[:], e4[:], mybir.ActivationFunctionType.Exp,
                                 bias=0.0, scale=-1.0)

            t1 = fixed.tile([64, 1024], f32)
            nc.vector.tensor_add(t1[:], e4[:, 0:1024], e4[:, 1024:2048])
            t2 = fixed.tile([64, 512], f32)
            nc.vector.tensor_add(t2[:], t1[:, 0:512], t1[:, 512:1024])
            t3 = fixed.tile([64, 256], f32)
            nc.vector.tensor_add(t3[:], t2[:, 0:256], t2[:, 256:512])
            s4 = fixed.tile([64, 256], f32)
            nc.vector.tensor_add(s4[:], t3[:], e4[:, 2048:2304])
            rs = fixed.tile([64, 256], f32)
            nc.vector.tensor_scalar_mul(rs[:], s4[:], 1.0 / 9.0)
            rr = fixed.tile([64, 256], f32)
            nc.vector.reciprocal(rr[:], rs[:])  # rr = 9/sum

            wf4 = fixed.tile([64, 9 * 256], f32 if GEMM_MODE != "bf16"
                             else mybir.dt.bfloat16)
            for k in range(9):
                nc.vector.tensor_mul(wf4[:, k * 256:(k + 1) * 256],
                                     e4[:, k * 256:(k + 1) * 256], rr[:])

            # k-on-partitions layout for the selector matmuls
            wf = fixed.tile([9, L], wf4.dtype)
            for k in range(9):
                src = wf4[:, k * 256:(k + 1) * 256]
                nc.sync.dma_start(wf[k:k + 1, :], src)

            # ---- main loop --------------------------------------------
            gdt_wf = gdt if GEMM_MODE != "fp32" else f32

            def xa_view(base_part, npart, off):
                v = xpadA[base_part:base_part + npart, off:off + 3 * PADW + W]
                return bass.AP(v.tensor, v.offset,
                               [v.ap[0], [PADW, 4], [1, W]])

            for t in range(NTILES):
                col = t * 512
                h0 = col // W  # = t*4
                pwf_tiles = []
                for m in range(5):
                    pwf = pwf_pool.tile([128, 512], f32, name=f"pwf{m}_{t}",
                                        tag="pwf")
                    rhs = wf[:, col:col + 512]
                    if GEMM_MODE == "fp32r":
                        rhs = rhs.bitcast(mybir.dt.float32r)
                    nc.tensor.matmul(pwf[:], ssel_sb[:, m * 128:(m + 1) * 128],
                                     rhs, start=True, stop=True)
                    pwf_tiles.append(pwf)

                filt_tiles = []
                for m, (k0, k1) in enumerate(PAIRS):
                    filt = filt_pool.tile([128, 512], gdt_wf,
                                          name=f"filt{m}_{t}", tag="filt")
                    pwf = pwf_tiles[m]
                    pv = pwf[:].rearrange("p (a b) -> p a b", b=128)
                    fv = filt.rearrange("p (a b) -> p a b", b=128)
                    if m == 3:  # (2,5): two half-ops
                        nc.vector.tensor_mul(
                            fv[0:64], xa_view(0, 64, KOFF[2] + h0 * PADW),
                            pv[0:64])
                        nc.vector.tensor_mul(
                            fv[64:128], xa_view(64, 64, KOFF[5] - 1 + h0 * PADW),
                            pv[64:128])
                    else:
                        nc.vector.tensor_mul(
                            fv[:], xa_view(0, 128, KOFF[k0] + h0 * PADW), pv[:])
                    filt_tiles.append(filt)

                pout = pout_pool.tile([128, 512], f32, name=f"pout_{t}",
                                      tag="pout")
                for m in range(5):
                    rhs = filt_tiles[m][:]
                    if GEMM_MODE == "fp32r":
                        rhs = rhs.bitcast(mybir.dt.float32r)
                    lhsT = wk_sb[:, m * COUT:(m + 1) * COUT]
                    if GEMM_MODE == "fp32r":
                        lhsT = lhsT  # already float32r dtype
                    nc.tensor.matmul(pout[:], lhsT, rhs,
                                     start=(m == 0), stop=(m == 4))

                osb = osb_pool.tile([COUT, 512], f32, name=f"osb_{t}", tag="osb")
                nc.scalar.activation(osb[:], pout[:],
                                     mybir.ActivationFunctionType.Identity,
                                     bias=bias_sb[:], scale=1.0)
                nc.sync.dma_start(out_dram.ap()[:, col:col + 512], osb[:])

          if reps == 1:
              body()
          else:
              with tc.For_i(0, reps, 1):
                  body()

    nc.compile()
    return nc


def host_weights(weights):
    """Host-side reorder of the (1, 576, 1, 128) weights into GEMM chunks."""
    w2 = np.ascontiguousarray(weights, dtype=np.float32).reshape(COUT, C * 9)
    wk = np.zeros((5, 128, COUT), np.float32)
    ssel = np.zeros((5, 9, 128), np.float32)
    cs = np.arange(C)
    for m, (k0, k1) in enumerate(PAIRS):
        wk[m, 0:64, :] = w2[:, cs * 9 + k0].T
        ssel[m, k0, 0:64] = 1.0
        if k1 is not None:
            wk[m, 64:128, :] = w2[:, cs * 9 + k1].T
            ssel[m, k1, 64:128] = 1.0
    return wk, ssel


def _cast_gemm(a):
    if GEMM_MODE == "bf16":
        import ml_dtypes
        return a.astype(ml_dtypes.bfloat16)
    return a.astype(np.float32)


_NC_CACHE = {}


def get_nc():
    key = GEMM_MODE
    if key not in _NC_CACHE:
        _NC_CACHE[key] = build_nc()
    return _NC_CACHE[key]


def kernel(input, g_buffer, weights, bias):
    input = np.asarray(input, dtype=np.float32)
    g_buffer = np.asarray(g_buffer, dtype=np.float32)
    weights = np.asarray(weights, dtype=np.float32)
    bias = np.asarray(bias, dtype=np.float32)

    wk, ssel = host_weights(weights)
    wk_c, ssel_c = _cast_gemm(wk), _cast_gemm(ssel)

    nc = get_nc()
    in_maps = []
    for b in range(N_CORES):
        in_maps.append({
            "x": np.ascontiguousarray(input[b]),
            "d": np.ascontiguousarray(g_buffer[b, 1]),
            "wk": wk_c,
            "ssel": ssel_c,
            "bias": bias,
        })
    res = run_bass_kernel_spmd(nc, in_maps, list(range(N_CORES)))
    out = np.stack([res.results[b]["out"].reshape(COUT, H, W)
                    for b in range(N_CORES)])
    return out
